# revision 15
# baseline (speedup 1.0000x reference)
"""DGCN kernel for Trainium2 (8 NeuronCores, data-parallel over batch).

Reference computation (per sample):
  h   = conv1x1(x)                                   # [C,N,T]
  hsum= h.sum(T)                                     # = W @ x.sum(T) + T*b
  a1  = softmax(relu(hsum.T @ memory * s))           # [N,N]
  a2  = softmax(relu(hsum.T @ hsum * s))             # [N,N]
  adj = softmax(fc_w0*a1 + fc_w1*a2 + fc_b)          # [N,N]
  adj = topk_mask(adj, K) * adj                      # keep K largest/row,
                                                     # ties -> lowest index
  g1  = h  (.) adj ; g2 = g1 (.) adj                 # node contraction
  z   = gcn_w @ [g1;g2] + gcn_b
  out = z*emb + x

Top-k trick: the softmax rows have a huge tie group at the "floor" value
(entries whose relus are all exactly 0 collapse to one float). The K-th
largest always lands inside it, so the threshold equals the floor value,
computed EXACTLY by pushing a virtual 884th zero-padded column through
the identical pipeline. Mask reproduces lowest-index-first tie breaking
via a prefix-count scan.

Layout/precision strategy (v2):
  - conv emitted as per-t-plane matmuls (lhsT = x[:, :, t] slices) so h
    lands directly node-major (hT8, fp8 x8) -- no stage-A transposes.
  - adjacency pipeline in bf16 matmuls (softmax floor trick survives:
    zero columns stay exactly zero); adj stored fp8 x256, pair-packed
    for DoubleRow.
  - diffusion g1 = h.adj via DoubleRow fp8 t-plane matmuls (256-deep
    contraction, 2x throughput) -> g1c (channel-major bf16, true scale).
  - one PE-transpose set g1c -> g1T8 (fp8 x8, pair-packed) feeds the
    second diffusion step, also DoubleRow fp8 -> g2c channel-major.
  - projection in bf16 with emb folded into the weights host-side;
    skip/bias folded into the x2 tile; output fp32.
  - samples software-pipelined: per iteration emit
    A(s), D0(s-1), T(s-1), B(s), D1(s-1), proj(s-1)
    so vector-heavy adjacency work overlaps PE-heavy diffusion.
"""
import math

import ml_dtypes
import numpy as np

import concourse.bass as bass
import concourse.mybir as mybir
import concourse.tile as tile
from concourse import bacc
from concourse.bass_utils import run_bass_kernel_spmd
from concourse.masks import make_identity

B, C, N, T = 32, 128, 883, 12
K = int(N * 0.8)  # 706
NCORES = 8
SPC = B // NCORES  # samples per core
SCALE = 1.0 / math.sqrt(C)
F32 = mybir.dt.float32
BF16 = mybir.dt.bfloat16
FP8 = mybir.dt.float8e4
AX = mybir.AxisListType
OP = mybir.AluOpType
ACTF = mybir.ActivationFunctionType
DR = mybir.MatmulPerfMode.DoubleRow
import os
USE_DR = os.environ.get("USE_DR", "1") != "0"
DBG = os.environ.get("DBG", "0") == "1"
DT8 = mybir.dt.float8e4 if os.environ.get("DT8", "fp8") == "fp8" else mybir.dt.bfloat16

NCH = (N + 127) // 128  # 7 node chunks
CH = [(j * 128, min(128, N - j * 128)) for j in range(NCH)]  # (start, size)
NPR = 4  # chunk pairs for fp8 DoubleRow (pair 3 slot 1 is zero padding)
MCH = [(0, 512), (512, 372)]  # (N+1)-wide adjacency free chunks
DCH = [(0, 512), (512, 371)]  # N-wide diffusion free chunks
CT = C * T  # 1536
S_H = 8.0  # fp8 scale for h / g1
S_A = 256.0  # fp8 scale for adj


def _fch(total, step=512):
    return [(f, min(step, total - f)) for f in range(0, total, step)]


def build_nc():
    nc = bacc.Bacc(None)
    x_d = nc.dram_tensor("x", [SPC, C, N, T], F32, kind="ExternalInput")
    y_d = nc.dram_tensor("y", [SPC, C, N, T], F32, kind="ExternalOutput")
    convwTb_d = nc.dram_tensor("convwTb", [C, C], BF16, kind="ExternalInput")
    biasC8_d = nc.dram_tensor("biasC8", [C, 512], F32, kind="ExternalInput")
    convb12p_d = nc.dram_tensor("convb12p", [C, 1], F32, kind="ExternalInput")
    memb_d = nc.dram_tensor("memb", [C, N + 1], BF16, kind="ExternalInput")
    fcw0_d = nc.dram_tensor("fcw0", [C, 1], F32, kind="ExternalInput")
    fcw1_d = nc.dram_tensor("fcw1", [C, 1], F32, kind="ExternalInput")
    fcb_d = nc.dram_tensor("fcb", [C, 1], F32, kind="ExternalInput")
    gw1Te_d = nc.dram_tensor("gw1Te", [C, C], BF16, kind="ExternalInput")
    gw2Te_d = nc.dram_tensor("gw2Te", [C, C], BF16, kind="ExternalInput")
    gbe_d = nc.dram_tensor("gbe", [C, 1], F32, kind="ExternalInput")
    if DBG:
        hT8_dbg = nc.dram_tensor("hT8_dbg", [SPC, C, NPR, 2, T, C], DT8, kind="ExternalOutput")
        xs_dbg = nc.dram_tensor("xs_dbg", [SPC, C, N + 1], BF16, kind="ExternalOutput")
        hs_dbg = nc.dram_tensor("hs_dbg", [SPC, C, N + 1], BF16, kind="ExternalOutput")
        adjP_dbg = nc.dram_tensor("adjP_dbg", [SPC, C, NPR, 2, N], DT8, kind="ExternalOutput")
        g1c_dbg = nc.dram_tensor("g1c_dbg", [SPC, C, N, T], BF16, kind="ExternalOutput")
        g2c_dbg = nc.dram_tensor("g2c_dbg", [SPC, C, N, T], BF16, kind="ExternalOutput")

    with tile.TileContext(nc) as tc:
        with (
            tc.tile_pool(name="const", bufs=1) as constp,
            tc.tile_pool(name="pers", bufs=2) as pers,
            tc.tile_pool(name="gper", bufs=1) as gper,
            tc.tile_pool(name="xin", bufs=2) as xinp,
            tc.tile_pool(name="x2in", bufs=2) as x2p,
            tc.tile_pool(name="scr", bufs=8 if DT8 == FP8 else 4) as scrp,
            tc.tile_pool(name="col", bufs=8) as colp,
            tc.tile_pool(name="outw", bufs=2) as outwp,
            tc.tile_pool(name="mmps", bufs=6, space=bass.MemorySpace.PSUM) as mmps,
            tc.tile_pool(name="tpps", bufs=2, space=bass.MemorySpace.PSUM) as tpps,
        ):
            # ---- constants / weights ----
            identb = constp.tile([128, 128], BF16)
            make_identity(nc, identb[:])
            zeros = constp.tile([128, N], F32)
            nc.gpsimd.memset(zeros[:], 0.0)
            convwTb = constp.tile_from(convwTb_d[:])
            biasC8 = constp.tile_from(biasC8_d[:])
            convb12p = constp.tile_from(convb12p_d[:])
            memb = constp.tile_from(memb_d[:])
            fcw0 = constp.tile_from(fcw0_d[:])
            fcw1 = constp.tile_from(fcw1_d[:])
            fcb = constp.tile_from(fcb_d[:])
            gw1Te = constp.tile_from(gw1Te_d[:])
            gw2Te = constp.tile_from(gw2Te_d[:])
            gbe = constp.tile_from(gbe_d[:])

            def stage_A(s):
                """conv t-planes -> hT8 (fp8 x8, pair-packed); xsumb; hsumb."""
                hT8 = pers.tile([128, NPR, 2, T, C], DT8, tag="hT8")
                xsumb = pers.tile([128, N + 1], BF16, tag="xsumb")
                hsumb = pers.tile([128, N + 1], BF16, tag="hsumb")
                # zero fp8 padding (pair 3 slot 1 fully; chunk-6 partitions)
                nc.gpsimd.memset(hT8[:, NPR - 1, 1], 0.0)
                nc.gpsimd.memset(hT8[96:, NPR - 1, 0], 0.0)
                nc.vector.memset(xsumb[:, N : N + 1], 0.0)

                xf = x_d[s].rearrange("c n t -> c (n t)")
                for j, (n0, sz) in enumerate(CH):
                    xb = xinp.tile([128, CT], BF16, tag="xb")
                    nc.gpsimd.dma_start(
                        xb[:, : sz * T], xf[:, n0 * T : (n0 + sz) * T]
                    )
                    xv = xb[:, : sz * T].rearrange("p (n t) -> p n t", t=T)
                    with nc.allow_low_precision(reason="bf16 xsum validated"):
                        nc.vector.tensor_reduce(
                            xsumb[:, n0 : n0 + sz], xv, axis=AX.X, op=OP.add
                        )
                    for tg in range(3):  # 4 t-planes per PSUM tile
                        ps = mmps.tile([128, 512], F32, tag="mm")
                        for tt in range(4):
                            t = tg * 4 + tt
                            nc.tensor.matmul(
                                ps[:sz, tt * 128 : (tt + 1) * 128],
                                xv[:, :, t], convwTb[:],
                                start=True, stop=True,
                            )
                        dst = hT8[:sz, j // 2, j % 2, tg * 4 : tg * 4 + 4, :]
                        dv = dst.rearrange("p t c -> p (t c)")
                        nc.vector.scalar_tensor_tensor(
                            dv, ps[:sz], S_H, biasC8[:sz],
                            op0=OP.mult, op1=OP.add,
                        )

                # hsum = W @ xsum + T*conv_b (bf16; virtual col stays 0)
                for f0, fs in MCH:
                    ps = mmps.tile([128, 512], F32, tag="mm")
                    nc.tensor.matmul(
                        ps[:, :fs], convwTb[:], xsumb[:, f0 : f0 + fs],
                        start=True, stop=True,
                    )
                    real = min(fs, N - f0)  # exclude virtual col from bias
                    nc.vector.tensor_scalar(
                        hsumb[:, f0 : f0 + real], ps[:, :real], convb12p[:],
                        None, op0=OP.add,
                    )
                    if real < fs:
                        nc.scalar.activation(
                            hsumb[:, f0 + real : f0 + fs], ps[:, real:fs],
                            ACTF.Copy,
                        )
                if DBG:
                    nc.sync.dma_start(hT8_dbg[s], hT8[:])
                    nc.sync.dma_start(xs_dbg[s], xsumb[:])
                    nc.sync.dma_start(hs_dbg[s], hsumb[:])
                return hT8, xsumb, hsumb

            def stage_B(s, hsumb):
                """adjacency + exact top-k mask -> adjP fp8 x256 pair-packed"""
                adjP = pers.tile([128, NPR, 2, N], DT8, tag="adjP")
                nc.gpsimd.memset(adjP[:, NPR - 1, 1], 0.0)
                nc.gpsimd.memset(adjP[96:, NPR - 1, 0], 0.0)

                for j, (n0, sz) in enumerate(CH):
                    lhs = hsumb[:, n0 : n0 + sz]
                    r1 = scrp.tile([128, N + 1], F32, tag="scr")
                    r2 = scrp.tile([128, N + 1], F32, tag="scr")
                    for (f0, fs), rt, rhs in (
                        (MCH[0], r1, memb), (MCH[1], r1, memb),
                        (MCH[0], r2, hsumb), (MCH[1], r2, hsumb),
                    ):
                        ps = mmps.tile([128, 512], F32, tag="mm")
                        nc.tensor.matmul(
                            ps[:sz, :fs], lhs, rhs[:, f0 : f0 + fs],
                            start=True, stop=True,
                        )
                        # relu(s * scale) -- matches reference op order
                        nc.scalar.activation(
                            rt[:sz, f0 : f0 + fs], ps[:sz, :fs], ACTF.Relu,
                            scale=SCALE,
                        )

                    def softmax_ext(rin, sz=sz):
                        """in-place softmax over cols [0,N); col N rides along"""
                        mn = colp.tile([128, 1], F32, tag="mn")
                        nc.vector.tensor_reduce(
                            mn[:sz], rin[:sz, :N], axis=AX.X, op=OP.max,
                            negate=True,
                        )
                        acc = colp.tile([128, 1], F32, tag="acc")
                        nc.scalar.activation(
                            rin[:sz], rin[:sz], ACTF.Exp,
                            bias=mn[:sz], scale=1.0, accum_out=acc[:sz],
                        )
                        zf = colp.tile([128, 1], F32, tag="zf")
                        nc.vector.tensor_sub(zf[:sz], acc[:sz], rin[:sz, N : N + 1])
                        nc.gpsimd.normalize_recip(rin[:sz], rin[:sz], zf[:sz])

                    softmax_ext(r1)  # r1 -> a1
                    softmax_ext(r2)  # r2 -> a2
                    # l = (fcw0*a1 + fcw1*a2) + fcb  -- reference association
                    t2 = scrp.tile([128, N + 1], F32, tag="scr")
                    nc.scalar.activation(
                        t2[:sz], r2[:sz], ACTF.Copy, scale=fcw1[:sz]
                    )
                    nc.vector.scalar_tensor_tensor(
                        r1[:sz], r1[:sz], fcw0[:sz], t2[:sz],
                        op0=OP.mult, op1=OP.add,
                    )
                    # (fcb add dropped: softmax is shift-invariant and the
                    # uniform shift preserves tie-group equality)
                    softmax_ext(r1)  # r1 -> adj
                    adj = r1
                    thr = adj[:sz, N : N + 1]
                    # ---- top-k mask, lowest-index tie breaking ----
                    gt = t2  # reuse
                    cnt = colp.tile([128, 1], F32, tag="cnt")
                    nc.vector.tensor_scalar(
                        gt[:sz, :N], adj[:sz, :N], thr, 0.0,
                        op0=OP.is_gt, op1=OP.add, accum_out=cnt[:sz],
                    )
                    eq = r2  # reuse
                    nc.vector.tensor_scalar(
                        eq[:sz, :N], adj[:sz, :N], thr, None, op0=OP.is_equal
                    )
                    # cum = cnt + prefix(eq); keep tie entries while cum <= K
                    cum = scrp.tile([128, N + 1], F32, tag="scr")
                    nc.vector.tensor_tensor_scan(
                        cum[:sz, :N], eq[:sz, :N], zeros[:sz, :N],
                        initial=cnt[:sz], op0=OP.add, op1=OP.add,
                    )
                    # eq <- (cum <= K)*eq ; then eq <- eq + gt
                    nc.gpsimd.tensor_scalar(
                        cum[:sz, :N], cum[:sz, :N], float(K), None, op0=OP.is_le
                    )
                    nc.gpsimd.tensor_mul(eq[:sz, :N], eq[:sz, :N], cum[:sz, :N])
                    nc.gpsimd.tensor_add(eq[:sz, :N], eq[:sz, :N], gt[:sz, :N])
                    # adjP = (adj * S_A) * mask, fp8
                    nc.vector.scalar_tensor_tensor(
                        adjP[:sz, j // 2, j % 2, :], adj[:sz, :N], S_A,
                        eq[:sz, :N], op0=OP.mult, op1=OP.mult,
                    )
                if DBG:
                    nc.sync.dma_start(adjP_dbg[s], adjP[:])
                return adjP

            def scale_evict(i, dst, src, scl):
                if i % 2 == 0:
                    nc.vector.tensor_scalar(dst, src, scl, None, op0=OP.mult)
                else:
                    nc.scalar.activation(dst, src, ACTF.Copy, scale=scl)

            def stage_D(hT8, adjP, dstc, ei):
                """one diffusion step: DoubleRow fp8 t-planes -> dstc bf16
                (channel-major [128, N, T], true scale)"""
                for t in range(T):
                    for m0, ms in DCH:
                        ps = mmps.tile([128, 512], F32, tag="mm")
                        if USE_DR:
                            for jj in range(NPR):
                                nc.tensor.matmul(
                                    ps[:, :ms],
                                    hT8[:, jj, :, t, :],
                                    adjP[:, jj, :, m0 : m0 + ms],
                                    start=(jj == 0), stop=(jj == NPR - 1),
                                    perf_mode=DR,
                                )
                        else:
                            for jc in range(NCH):
                                nc.tensor.matmul(
                                    ps[:, :ms],
                                    hT8[:, jc // 2, jc % 2, t, :],
                                    adjP[:, jc // 2, jc % 2, m0 : m0 + ms],
                                    start=(jc == 0), stop=(jc == NCH - 1),
                                )
                        scale_evict(
                            ei + t * 2 + (m0 > 0),
                            dstc[:, m0 : m0 + ms, t], ps[:, :ms],
                            1.0 / (S_H * S_A),
                        )

            def stage_T(g1c, g1T8):
                """transpose g1c -> g1T8 (fp8 x8, pair-packed)"""
                nc.gpsimd.memset(g1T8[:, NPR - 1, 1], 0.0)
                nc.gpsimd.memset(g1T8[96:, NPR - 1, 0], 0.0)
                for kk, (m0, msz) in enumerate(CH):
                    for tg in range(3):
                        tp = tpps.tile([128, 4, 128], BF16, tag="tp")
                        for tt in range(4):
                            t = tg * 4 + tt
                            nc.tensor.transpose(
                                tp[:msz, tt, :], g1c[:, m0 : m0 + msz, t],
                                identb[:],
                            )
                        dst = g1T8[:msz, kk // 2, kk % 2, tg * 4 : tg * 4 + 4, :]
                        scale_evict(
                            kk + tg,
                            dst.rearrange("p t c -> p (t c)"),
                            tp[:msz].rearrange("p t c -> p (t c)"), S_H,
                        )

            def stage_P(s, g1c, g2c):
                """projection (emb folded into weights) + skip + output DMA"""
                if DBG:
                    nc.sync.dma_start(g1c_dbg[s], g1c[:])
                    nc.sync.dma_start(g2c_dbg[s], g2c[:])
                xf = x_d[s].rearrange("c n t -> c (n t)")
                yf = y_d[s].rearrange("c n t -> c (n t)")
                g1f = g1c.rearrange("p n t -> p (n t)")
                g2f = g2c.rearrange("p n t -> p (n t)")
                for j, (n0, sz) in enumerate(CH):
                    ow = outwp.tile([128, CT], F32, tag="ow")
                    x2 = x2p.tile([128, CT], F32, tag="x2")
                    nc.sync.dma_start(
                        x2[:, : sz * T], xf[:, n0 * T : (n0 + sz) * T]
                    )
                    for f0, fs in _fch(sz * T):
                        ps = mmps.tile([128, 512], F32, tag="mm")
                        nc.tensor.matmul(
                            ps[:, :fs], gw1Te[:],
                            g1f[:, n0 * T + f0 : n0 * T + f0 + fs],
                            start=True, stop=False,
                        )
                        nc.tensor.matmul(
                            ps[:, :fs], gw2Te[:],
                            g2f[:, n0 * T + f0 : n0 * T + f0 + fs],
                            start=False, stop=True,
                        )
                        nc.vector.scalar_tensor_tensor(
                            ow[:, f0 : f0 + fs], ps[:, :fs], gbe[:],
                            x2[:, f0 : f0 + fs], op0=OP.add, op1=OP.add,
                        )
                    nc.sync.dma_start(yf[:, n0 * T : (n0 + sz) * T], ow[:, : sz * T])

            # ---- software-pipelined main loop ----
            prev = None
            for s in range(SPC):
                hT8, xsumb, hsumb = stage_A(s)
                if prev is not None:
                    ps_, hT8_, adjP_ = prev
                    g1c = gper.tile([128, N, T], BF16, tag="g1c")
                    g2c = gper.tile([128, N, T], BF16, tag="g2c")
                    g1T8 = gper.tile([128, NPR, 2, T, C], DT8, tag="g1T8")
                    stage_D(hT8_, adjP_, g1c, 0)
                    stage_T(g1c, g1T8)
                adjP = stage_B(s, hsumb)
                if prev is not None:
                    stage_D(g1T8, adjP_, g2c, 1)
                    stage_P(ps_, g1c, g2c)
                prev = (s, hT8, adjP)

            ps_, hT8_, adjP_ = prev
            g1c = gper.tile([128, N, T], BF16, tag="g1c")
            g2c = gper.tile([128, N, T], BF16, tag="g2c")
            g1T8 = gper.tile([128, NPR, 2, T, C], DT8, tag="g1T8")
            stage_D(hT8_, adjP_, g1c, 0)
            stage_T(g1c, g1T8)
            stage_D(g1T8, adjP_, g2c, 1)
            stage_P(ps_, g1c, g2c)
    nc.compile()
    return nc


_NC = None


def _get_nc():
    global _NC
    if _NC is None:
        _NC = build_nc()
    return _NC


def make_in_maps(inputs):
    x = np.ascontiguousarray(np.asarray(inputs["x"], dtype=np.float32))
    conv_w = np.asarray(inputs["conv_w"], np.float32)
    conv_b = np.asarray(inputs["conv_b"], np.float32)
    memory = np.asarray(inputs["memory"], np.float32)
    fc_w = np.asarray(inputs["fc_w"], np.float32)
    fc_b = np.asarray(inputs["fc_b"], np.float32)
    gcn_w = np.asarray(inputs["gcn_w"], np.float32)
    gcn_b = np.asarray(inputs["gcn_b"], np.float32)
    emb = np.asarray(inputs["emb"], np.float32).reshape(C)

    membx = np.zeros((C, N + 1), np.float32)
    membx[:, :N] = memory
    shared = {
        "convwTb": np.ascontiguousarray(conv_w.T).astype(ml_dtypes.bfloat16),
        "biasC8": np.tile(S_H * conv_b[None, :], (C, 4)).astype(np.float32),
        "convb12p": (T * conv_b).reshape(C, 1).copy(),
        "memb": membx.astype(ml_dtypes.bfloat16),
        "fcw0": np.full((C, 1), fc_w[0, 0], np.float32),
        "fcw1": np.full((C, 1), fc_w[0, 1], np.float32),
        "fcb": np.full((C, 1), fc_b[0], np.float32),
        "gw1Te": np.ascontiguousarray(
            (gcn_w[:, :C] * emb[:, None]).T
        ).astype(ml_dtypes.bfloat16),
        "gw2Te": np.ascontiguousarray(
            (gcn_w[:, C:] * emb[:, None]).T
        ).astype(ml_dtypes.bfloat16),
        "gbe": (gcn_b * emb).reshape(C, 1).astype(np.float32),
    }
    return [
        {"x": np.ascontiguousarray(x[c * SPC : (c + 1) * SPC]), **shared}
        for c in range(NCORES)
    ]


def kernel(**inputs) -> np.ndarray:
    nc = _get_nc()
    in_maps = make_in_maps(inputs)
    res = run_bass_kernel_spmd(nc, in_maps, list(range(NCORES)))
    outs = [res.results[c]["y"] for c in range(NCORES)]
    return np.concatenate(outs, axis=0).astype(np.float32)


# revision 16
# speedup vs baseline: 1.0210x; 1.0210x over previous
"""DGCN kernel for Trainium2 (8 NeuronCores, data-parallel over batch).

Reference computation (per sample):
  h   = conv1x1(x)                                   # [C,N,T]
  hsum= h.sum(T)                                     # = W @ x.sum(T) + T*b
  a1  = softmax(relu(hsum.T @ memory * s))           # [N,N]
  a2  = softmax(relu(hsum.T @ hsum * s))             # [N,N]
  adj = softmax(fc_w0*a1 + fc_w1*a2 + fc_b)          # [N,N]
  adj = topk_mask(adj, K) * adj                      # keep K largest/row,
                                                     # ties -> lowest index
  g1  = h  (.) adj ; g2 = g1 (.) adj                 # node contraction
  z   = gcn_w @ [g1;g2] + gcn_b
  out = z*emb + x

Top-k trick: the softmax rows have a huge tie group at the "floor" value
(entries whose relus are all exactly 0 collapse to one float). The K-th
largest always lands inside it, so the threshold equals the floor value,
computed EXACTLY by pushing a virtual 884th zero-padded column through
the identical pipeline. Mask reproduces lowest-index-first tie breaking
via a prefix-count scan.

Layout/precision strategy (v2):
  - conv emitted as per-t-plane matmuls (lhsT = x[:, :, t] slices) so h
    lands directly node-major (hT8, fp8 x8) -- no stage-A transposes.
  - adjacency pipeline in bf16 matmuls (softmax floor trick survives:
    zero columns stay exactly zero); adj stored fp8 x256, pair-packed
    for DoubleRow.
  - diffusion g1 = h.adj via DoubleRow fp8 t-plane matmuls (256-deep
    contraction, 2x throughput) -> g1c (channel-major bf16, true scale).
  - one PE-transpose set g1c -> g1T8 (fp8 x8, pair-packed) feeds the
    second diffusion step, also DoubleRow fp8 -> g2c channel-major.
  - projection in bf16 with emb folded into the weights host-side;
    skip/bias folded into the x2 tile; output fp32.
  - samples software-pipelined: per iteration emit
    A(s), D0(s-1), T(s-1), B(s), D1(s-1), proj(s-1)
    so vector-heavy adjacency work overlaps PE-heavy diffusion.
"""
import math

import ml_dtypes
import numpy as np

import concourse.bass as bass
import concourse.mybir as mybir
import concourse.tile as tile
from concourse import bacc
from concourse.bass_utils import run_bass_kernel_spmd
from concourse.masks import make_identity

B, C, N, T = 32, 128, 883, 12
K = int(N * 0.8)  # 706
NCORES = 8
SPC = B // NCORES  # samples per core
SCALE = 1.0 / math.sqrt(C)
F32 = mybir.dt.float32
BF16 = mybir.dt.bfloat16
FP8 = mybir.dt.float8e4
AX = mybir.AxisListType
OP = mybir.AluOpType
ACTF = mybir.ActivationFunctionType
DR = mybir.MatmulPerfMode.DoubleRow
import os
USE_DR = os.environ.get("USE_DR", "1") != "0"
DBG = os.environ.get("DBG", "0") == "1"
DT8 = mybir.dt.float8e4 if os.environ.get("DT8", "fp8") == "fp8" else mybir.dt.bfloat16

NCH = (N + 127) // 128  # 7 node chunks
CH = [(j * 128, min(128, N - j * 128)) for j in range(NCH)]  # (start, size)
NPR = 4  # chunk pairs for fp8 DoubleRow (pair 3 slot 1 is zero padding)
MCH = [(0, 512), (512, 372)]  # (N+1)-wide adjacency free chunks
DCH = [(0, 512), (512, 371)]  # N-wide diffusion free chunks
NP8 = 896  # adjP padded row length (even stride for DoubleRow slots)
DCH_DR = [(0, 512, 512), (512, 384, 371)]  # (m0, padded ms, real ms)
CT = C * T  # 1536
S_H = 8.0  # fp8 scale for h / g1
S_A = 256.0  # fp8 scale for adj


def _fch(total, step=512):
    return [(f, min(step, total - f)) for f in range(0, total, step)]


def build_nc():
    nc = bacc.Bacc(None)
    x_d = nc.dram_tensor("x", [SPC, C, N, T], F32, kind="ExternalInput")
    y_d = nc.dram_tensor("y", [SPC, C, N, T], F32, kind="ExternalOutput")
    convwTb_d = nc.dram_tensor("convwTb", [C, C], BF16, kind="ExternalInput")
    biasC8_d = nc.dram_tensor("biasC8", [C, 512], F32, kind="ExternalInput")
    convb12p_d = nc.dram_tensor("convb12p", [C, 1], F32, kind="ExternalInput")
    memb_d = nc.dram_tensor("memb", [C, N + 1], BF16, kind="ExternalInput")
    fcw0_d = nc.dram_tensor("fcw0", [C, 1], F32, kind="ExternalInput")
    fcw1_d = nc.dram_tensor("fcw1", [C, 1], F32, kind="ExternalInput")
    fcb_d = nc.dram_tensor("fcb", [C, 1], F32, kind="ExternalInput")
    gw1Te_d = nc.dram_tensor("gw1Te", [C, C], BF16, kind="ExternalInput")
    gw2Te_d = nc.dram_tensor("gw2Te", [C, C], BF16, kind="ExternalInput")
    gbe_d = nc.dram_tensor("gbe", [C, 1], F32, kind="ExternalInput")
    if DBG:
        hT8_dbg = nc.dram_tensor("hT8_dbg", [SPC, C, NPR, 2, T, C], DT8, kind="ExternalOutput")
        xs_dbg = nc.dram_tensor("xs_dbg", [SPC, C, N + 1], BF16, kind="ExternalOutput")
        hs_dbg = nc.dram_tensor("hs_dbg", [SPC, C, N + 1], BF16, kind="ExternalOutput")
        adjP_dbg = nc.dram_tensor("adjP_dbg", [SPC, C, NPR, 2, N], DT8, kind="ExternalOutput")
        g1c_dbg = nc.dram_tensor("g1c_dbg", [SPC, C, N, T], BF16, kind="ExternalOutput")
        g2c_dbg = nc.dram_tensor("g2c_dbg", [SPC, C, N, T], BF16, kind="ExternalOutput")

    with tile.TileContext(nc) as tc:
        with (
            tc.tile_pool(name="const", bufs=1) as constp,
            tc.tile_pool(name="pers", bufs=2) as pers,
            tc.tile_pool(name="gper", bufs=1) as gper,
            tc.tile_pool(name="xin", bufs=2) as xinp,
            tc.tile_pool(name="x2in", bufs=2) as x2p,
            tc.tile_pool(name="scr", bufs=8 if DT8 == FP8 else 4) as scrp,
            tc.tile_pool(name="col", bufs=8) as colp,
            tc.tile_pool(name="outw", bufs=2) as outwp,
            tc.tile_pool(name="mmps", bufs=6, space=bass.MemorySpace.PSUM) as mmps,
            tc.tile_pool(name="tpps", bufs=2, space=bass.MemorySpace.PSUM) as tpps,
        ):
            # ---- constants / weights ----
            identb = constp.tile([128, 128], BF16)
            make_identity(nc, identb[:])
            zeros = constp.tile([128, N], F32)
            nc.gpsimd.memset(zeros[:], 0.0)
            convwTb = constp.tile_from(convwTb_d[:])
            biasC8 = constp.tile_from(biasC8_d[:])
            convb12p = constp.tile_from(convb12p_d[:])
            memb = constp.tile_from(memb_d[:])
            fcw0 = constp.tile_from(fcw0_d[:])
            fcw1 = constp.tile_from(fcw1_d[:])
            fcb = constp.tile_from(fcb_d[:])
            gw1Te = constp.tile_from(gw1Te_d[:])
            gw2Te = constp.tile_from(gw2Te_d[:])
            gbe = constp.tile_from(gbe_d[:])

            def stage_A(s):
                """conv t-planes -> hT8 (fp8 x8, pair-packed); xsumb; hsumb."""
                hT8 = pers.tile([128, NPR, 2, T, C], DT8, tag="hT8")
                xsumb = pers.tile([128, N + 1], BF16, tag="xsumb")
                hsumb = pers.tile([128, N + 1], BF16, tag="hsumb")
                # zero fp8 padding (pair 3 slot 1 fully; chunk-6 partitions)
                nc.gpsimd.memset(hT8[:, NPR - 1, 1], 0.0)
                nc.gpsimd.memset(hT8[96:, NPR - 1, 0], 0.0)
                nc.vector.memset(xsumb[:, N : N + 1], 0.0)

                xf = x_d[s].rearrange("c n t -> c (n t)")
                for j, (n0, sz) in enumerate(CH):
                    xb = xinp.tile([128, CT], BF16, tag="xb")
                    nc.gpsimd.dma_start(
                        xb[:, : sz * T], xf[:, n0 * T : (n0 + sz) * T]
                    )
                    xv = xb[:, : sz * T].rearrange("p (n t) -> p n t", t=T)
                    with nc.allow_low_precision(reason="bf16 xsum validated"):
                        nc.vector.tensor_reduce(
                            xsumb[:, n0 : n0 + sz], xv, axis=AX.X, op=OP.add
                        )
                    for tg in range(3):  # 4 t-planes per PSUM tile
                        ps = mmps.tile([128, 512], F32, tag="mm")
                        for tt in range(4):
                            t = tg * 4 + tt
                            nc.tensor.matmul(
                                ps[:sz, tt * 128 : (tt + 1) * 128],
                                xv[:, :, t], convwTb[:],
                                start=True, stop=True,
                            )
                        dst = hT8[:sz, j // 2, j % 2, tg * 4 : tg * 4 + 4, :]
                        dv = dst.rearrange("p t c -> p (t c)")
                        nc.vector.scalar_tensor_tensor(
                            dv, ps[:sz], S_H, biasC8[:sz],
                            op0=OP.mult, op1=OP.add,
                        )

                # hsum = W @ xsum + T*conv_b (bf16; virtual col stays 0)
                for f0, fs in MCH:
                    ps = mmps.tile([128, 512], F32, tag="mm")
                    nc.tensor.matmul(
                        ps[:, :fs], convwTb[:], xsumb[:, f0 : f0 + fs],
                        start=True, stop=True,
                    )
                    real = min(fs, N - f0)  # exclude virtual col from bias
                    nc.vector.tensor_scalar(
                        hsumb[:, f0 : f0 + real], ps[:, :real], convb12p[:],
                        None, op0=OP.add,
                    )
                    if real < fs:
                        nc.scalar.activation(
                            hsumb[:, f0 + real : f0 + fs], ps[:, real:fs],
                            ACTF.Copy,
                        )
                if DBG:
                    nc.sync.dma_start(hT8_dbg[s], hT8[:])
                    nc.sync.dma_start(xs_dbg[s], xsumb[:])
                    nc.sync.dma_start(hs_dbg[s], hsumb[:])
                return hT8, xsumb, hsumb

            def stage_B(s, hsumb):
                """adjacency + exact top-k mask -> adjP fp8 x256 pair-packed"""
                adjP = pers.tile([128, NPR, 2, NP8], DT8, tag="adjP")
                nc.gpsimd.memset(adjP[:, NPR - 1, 1], 0.0)
                nc.gpsimd.memset(adjP[96:, NPR - 1, 0], 0.0)
                nc.gpsimd.memset(adjP[:, :, :, N:], 0.0)

                for j, (n0, sz) in enumerate(CH):
                    lhs = hsumb[:, n0 : n0 + sz]
                    r1 = scrp.tile([128, N + 1], F32, tag="scr")
                    r2 = scrp.tile([128, N + 1], F32, tag="scr")
                    for (f0, fs), rt, rhs in (
                        (MCH[0], r1, memb), (MCH[1], r1, memb),
                        (MCH[0], r2, hsumb), (MCH[1], r2, hsumb),
                    ):
                        ps = mmps.tile([128, 512], F32, tag="mm")
                        nc.tensor.matmul(
                            ps[:sz, :fs], lhs, rhs[:, f0 : f0 + fs],
                            start=True, stop=True,
                        )
                        # relu(s * scale) -- matches reference op order
                        nc.scalar.activation(
                            rt[:sz, f0 : f0 + fs], ps[:sz, :fs], ACTF.Relu,
                            scale=SCALE,
                        )

                    def softmax_ext(rin, sz=sz):
                        """in-place softmax over cols [0,N); col N rides along"""
                        mn = colp.tile([128, 1], F32, tag="mn")
                        nc.vector.tensor_reduce(
                            mn[:sz], rin[:sz, :N], axis=AX.X, op=OP.max,
                            negate=True,
                        )
                        acc = colp.tile([128, 1], F32, tag="acc")
                        nc.scalar.activation(
                            rin[:sz], rin[:sz], ACTF.Exp,
                            bias=mn[:sz], scale=1.0, accum_out=acc[:sz],
                        )
                        zf = colp.tile([128, 1], F32, tag="zf")
                        nc.vector.tensor_sub(zf[:sz], acc[:sz], rin[:sz, N : N + 1])
                        nc.gpsimd.normalize_recip(rin[:sz], rin[:sz], zf[:sz])

                    softmax_ext(r1)  # r1 -> a1
                    softmax_ext(r2)  # r2 -> a2
                    # l = (fcw0*a1 + fcw1*a2) + fcb  -- reference association
                    t2 = scrp.tile([128, N + 1], F32, tag="scr")
                    nc.scalar.activation(
                        t2[:sz], r2[:sz], ACTF.Copy, scale=fcw1[:sz]
                    )
                    nc.vector.scalar_tensor_tensor(
                        r1[:sz], r1[:sz], fcw0[:sz], t2[:sz],
                        op0=OP.mult, op1=OP.add,
                    )
                    # (fcb add dropped: softmax is shift-invariant and the
                    # uniform shift preserves tie-group equality)
                    softmax_ext(r1)  # r1 -> adj
                    adj = r1
                    thr = adj[:sz, N : N + 1]
                    # ---- top-k mask, lowest-index tie breaking ----
                    gt = t2  # reuse
                    cnt = colp.tile([128, 1], F32, tag="cnt")
                    nc.vector.tensor_scalar(
                        gt[:sz, :N], adj[:sz, :N], thr, 0.0,
                        op0=OP.is_gt, op1=OP.add, accum_out=cnt[:sz],
                    )
                    eq = r2  # reuse
                    nc.vector.tensor_scalar(
                        eq[:sz, :N], adj[:sz, :N], thr, None, op0=OP.is_equal
                    )
                    # cum = cnt + prefix(eq); keep tie entries while cum <= K
                    cum = scrp.tile([128, N + 1], F32, tag="scr")
                    nc.vector.tensor_tensor_scan(
                        cum[:sz, :N], eq[:sz, :N], zeros[:sz, :N],
                        initial=cnt[:sz], op0=OP.add, op1=OP.add,
                    )
                    # eq <- (cum <= K)*eq ; then eq <- eq + gt
                    nc.gpsimd.tensor_scalar(
                        cum[:sz, :N], cum[:sz, :N], float(K), None, op0=OP.is_le
                    )
                    nc.gpsimd.tensor_mul(eq[:sz, :N], eq[:sz, :N], cum[:sz, :N])
                    nc.gpsimd.tensor_add(eq[:sz, :N], eq[:sz, :N], gt[:sz, :N])
                    # adjP = (adj * S_A) * mask, fp8
                    nc.vector.scalar_tensor_tensor(
                        adjP[:sz, j // 2, j % 2, :N], adj[:sz, :N], S_A,
                        eq[:sz, :N], op0=OP.mult, op1=OP.mult,
                    )
                if DBG:
                    nc.sync.dma_start(adjP_dbg[s], adjP[:])
                return adjP

            def scale_evict(i, dst, src, scl):
                if i % 2 == 0:
                    nc.vector.tensor_scalar(dst, src, scl, None, op0=OP.mult)
                else:
                    nc.scalar.activation(dst, src, ACTF.Copy, scale=scl)

            def stage_D(hT8, adjP, dstc, ei):
                """one diffusion step: DoubleRow fp8 t-planes -> dstc bf16
                (channel-major [128, N, T], true scale)"""
                for t in range(T):
                    for m0, msp, ms in DCH_DR:
                        ps = mmps.tile([128, 512], F32, tag="mm")
                        if USE_DR:
                            for jj in range(NPR):
                                nc.tensor.matmul(
                                    ps[:, :msp],
                                    hT8[:, jj, :, t, :],
                                    adjP[:, jj, :, m0 : m0 + msp],
                                    start=(jj == 0), stop=(jj == NPR - 1),
                                    perf_mode=DR,
                                )
                        else:
                            for jc in range(NCH):
                                nc.tensor.matmul(
                                    ps[:, :ms],
                                    hT8[:, jc // 2, jc % 2, t, :],
                                    adjP[:, jc // 2, jc % 2, m0 : m0 + ms],
                                    start=(jc == 0), stop=(jc == NCH - 1),
                                )
                        scale_evict(
                            ei + t * 2 + (m0 > 0),
                            dstc[:, m0 : m0 + ms, t], ps[:, :ms],
                            1.0 / (S_H * S_A),
                        )

            def stage_T(g1c, g1T8):
                """transpose g1c -> g1T8 (fp8 x8, pair-packed)"""
                nc.gpsimd.memset(g1T8[:, NPR - 1, 1], 0.0)
                nc.gpsimd.memset(g1T8[96:, NPR - 1, 0], 0.0)
                for kk, (m0, msz) in enumerate(CH):
                    for tg in range(3):
                        tp = tpps.tile([128, 4, 128], BF16, tag="tp")
                        for tt in range(4):
                            t = tg * 4 + tt
                            nc.tensor.transpose(
                                tp[:msz, tt, :], g1c[:, m0 : m0 + msz, t],
                                identb[:],
                            )
                        dst = g1T8[:msz, kk // 2, kk % 2, tg * 4 : tg * 4 + 4, :]
                        scale_evict(
                            kk + tg,
                            dst.rearrange("p t c -> p (t c)"),
                            tp[:msz].rearrange("p t c -> p (t c)"), S_H,
                        )

            def stage_P(s, g1c, g2c):
                """projection (emb folded into weights) + skip + output DMA"""
                if DBG:
                    nc.sync.dma_start(g1c_dbg[s], g1c[:])
                    nc.sync.dma_start(g2c_dbg[s], g2c[:])
                xf = x_d[s].rearrange("c n t -> c (n t)")
                yf = y_d[s].rearrange("c n t -> c (n t)")
                g1f = g1c.rearrange("p n t -> p (n t)")
                g2f = g2c.rearrange("p n t -> p (n t)")
                for j, (n0, sz) in enumerate(CH):
                    ow = outwp.tile([128, CT], F32, tag="ow")
                    x2 = x2p.tile([128, CT], F32, tag="x2")
                    nc.sync.dma_start(
                        x2[:, : sz * T], xf[:, n0 * T : (n0 + sz) * T]
                    )
                    for f0, fs in _fch(sz * T):
                        ps = mmps.tile([128, 512], F32, tag="mm")
                        nc.tensor.matmul(
                            ps[:, :fs], gw1Te[:],
                            g1f[:, n0 * T + f0 : n0 * T + f0 + fs],
                            start=True, stop=False,
                        )
                        nc.tensor.matmul(
                            ps[:, :fs], gw2Te[:],
                            g2f[:, n0 * T + f0 : n0 * T + f0 + fs],
                            start=False, stop=True,
                        )
                        nc.vector.scalar_tensor_tensor(
                            ow[:, f0 : f0 + fs], ps[:, :fs], gbe[:],
                            x2[:, f0 : f0 + fs], op0=OP.add, op1=OP.add,
                        )
                    nc.sync.dma_start(yf[:, n0 * T : (n0 + sz) * T], ow[:, : sz * T])

            # ---- software-pipelined main loop ----
            prev = None
            for s in range(SPC):
                hT8, xsumb, hsumb = stage_A(s)
                if prev is not None:
                    ps_, hT8_, adjP_ = prev
                    g1c = gper.tile([128, N, T], BF16, tag="g1c")
                    g2c = gper.tile([128, N, T], BF16, tag="g2c")
                    g1T8 = gper.tile([128, NPR, 2, T, C], DT8, tag="g1T8")
                    stage_D(hT8_, adjP_, g1c, 0)
                    stage_T(g1c, g1T8)
                adjP = stage_B(s, hsumb)
                if prev is not None:
                    stage_D(g1T8, adjP_, g2c, 1)
                    stage_P(ps_, g1c, g2c)
                prev = (s, hT8, adjP)

            ps_, hT8_, adjP_ = prev
            g1c = gper.tile([128, N, T], BF16, tag="g1c")
            g2c = gper.tile([128, N, T], BF16, tag="g2c")
            g1T8 = gper.tile([128, NPR, 2, T, C], DT8, tag="g1T8")
            stage_D(hT8_, adjP_, g1c, 0)
            stage_T(g1c, g1T8)
            stage_D(g1T8, adjP_, g2c, 1)
            stage_P(ps_, g1c, g2c)
    nc.compile()
    return nc


_NC = None


def _get_nc():
    global _NC
    if _NC is None:
        _NC = build_nc()
    return _NC


def make_in_maps(inputs):
    x = np.ascontiguousarray(np.asarray(inputs["x"], dtype=np.float32))
    conv_w = np.asarray(inputs["conv_w"], np.float32)
    conv_b = np.asarray(inputs["conv_b"], np.float32)
    memory = np.asarray(inputs["memory"], np.float32)
    fc_w = np.asarray(inputs["fc_w"], np.float32)
    fc_b = np.asarray(inputs["fc_b"], np.float32)
    gcn_w = np.asarray(inputs["gcn_w"], np.float32)
    gcn_b = np.asarray(inputs["gcn_b"], np.float32)
    emb = np.asarray(inputs["emb"], np.float32).reshape(C)

    membx = np.zeros((C, N + 1), np.float32)
    membx[:, :N] = memory
    shared = {
        "convwTb": np.ascontiguousarray(conv_w.T).astype(ml_dtypes.bfloat16),
        "biasC8": np.tile(S_H * conv_b[None, :], (C, 4)).astype(np.float32),
        "convb12p": (T * conv_b).reshape(C, 1).copy(),
        "memb": membx.astype(ml_dtypes.bfloat16),
        "fcw0": np.full((C, 1), fc_w[0, 0], np.float32),
        "fcw1": np.full((C, 1), fc_w[0, 1], np.float32),
        "fcb": np.full((C, 1), fc_b[0], np.float32),
        "gw1Te": np.ascontiguousarray(
            (gcn_w[:, :C] * emb[:, None]).T
        ).astype(ml_dtypes.bfloat16),
        "gw2Te": np.ascontiguousarray(
            (gcn_w[:, C:] * emb[:, None]).T
        ).astype(ml_dtypes.bfloat16),
        "gbe": (gcn_b * emb).reshape(C, 1).astype(np.float32),
    }
    return [
        {"x": np.ascontiguousarray(x[c * SPC : (c + 1) * SPC]), **shared}
        for c in range(NCORES)
    ]


def kernel(**inputs) -> np.ndarray:
    nc = _get_nc()
    in_maps = make_in_maps(inputs)
    res = run_bass_kernel_spmd(nc, in_maps, list(range(NCORES)))
    outs = [res.results[c]["y"] for c in range(NCORES)]
    return np.concatenate(outs, axis=0).astype(np.float32)


# revision 19
# speedup vs baseline: 1.7948x; 1.7579x over previous
"""DGCN kernel for Trainium2 (8 NeuronCores, data-parallel over batch).

Reference computation (per sample):
  h   = conv1x1(x)                                   # [C,N,T]
  hsum= h.sum(T)                                     # = W @ x.sum(T) + T*b
  a1  = softmax(relu(hsum.T @ memory * s))           # [N,N]
  a2  = softmax(relu(hsum.T @ hsum * s))             # [N,N]
  adj = softmax(fc_w0*a1 + fc_w1*a2 + fc_b)          # [N,N]
  adj = topk_mask(adj, K) * adj                      # keep K largest/row,
                                                     # ties -> lowest index
  g1  = h  (.) adj ; g2 = g1 (.) adj                 # node contraction
  z   = gcn_w @ [g1;g2] + gcn_b
  out = z*emb + x

Top-k trick: the softmax rows have a huge tie group at the "floor" value
(entries whose relus are all exactly 0 collapse to one float). The K-th
largest always lands inside it, so the threshold equals the floor value,
computed EXACTLY by pushing a virtual 884th zero-padded column through
the identical pipeline. Mask reproduces lowest-index-first tie breaking
via a prefix-count scan.

Layout/precision strategy (v2):
  - conv emitted as per-t-plane matmuls (lhsT = x[:, :, t] slices) so h
    lands directly node-major (hT8, fp8 x8) -- no stage-A transposes.
  - adjacency pipeline in bf16 matmuls (softmax floor trick survives:
    zero columns stay exactly zero); adj stored fp8 x256, pair-packed
    for DoubleRow.
  - diffusion g1 = h.adj via DoubleRow fp8 t-plane matmuls (256-deep
    contraction, 2x throughput) -> g1c (channel-major bf16, true scale).
  - one PE-transpose set g1c -> g1T8 (fp8 x8, pair-packed) feeds the
    second diffusion step, also DoubleRow fp8 -> g2c channel-major.
  - projection in bf16 with emb folded into the weights host-side;
    skip/bias folded into the x2 tile; output fp32.
  - samples software-pipelined: per iteration emit
    A(s), D0(s-1), T(s-1), B(s), D1(s-1), proj(s-1)
    so vector-heavy adjacency work overlaps PE-heavy diffusion.
"""
import math

import ml_dtypes
import numpy as np

import concourse.bass as bass
import concourse.mybir as mybir
import concourse.tile as tile
from concourse import bacc
from concourse.bass_utils import run_bass_kernel_spmd
from concourse.masks import make_identity

B, C, N, T = 32, 128, 883, 12
K = int(N * 0.8)  # 706
NCORES = 8
SPC = B // NCORES  # samples per core
SCALE = 1.0 / math.sqrt(C)
F32 = mybir.dt.float32
BF16 = mybir.dt.bfloat16
FP8 = mybir.dt.float8e4
AX = mybir.AxisListType
OP = mybir.AluOpType
ACTF = mybir.ActivationFunctionType
DR = mybir.MatmulPerfMode.DoubleRow
import os
USE_DR = os.environ.get("USE_DR", "1") != "0"
DBG = os.environ.get("DBG", "0") == "1"
DT8 = mybir.dt.float8e4 if os.environ.get("DT8", "fp8") == "fp8" else mybir.dt.bfloat16

NCH = (N + 127) // 128  # 7 node chunks
CH = [(j * 128, min(128, N - j * 128)) for j in range(NCH)]  # (start, size)
NPR = 4  # chunk pairs for fp8 DoubleRow (pair 3 slot 1 is zero padding)
MCH = [(0, 512), (512, 372)]  # (N+1)-wide adjacency free chunks
DCH = [(0, 512), (512, 371)]  # N-wide diffusion free chunks
NP8 = 896  # adjP padded row length (even stride for DoubleRow slots)
DCH_DR = [(0, 512, 512), (512, 384, 371)]  # (m0, padded ms, real ms)
CT = C * T  # 1536
S_H = 8.0  # fp8 scale for h / g1
S_A = 256.0  # fp8 scale for adj


def _fch(total, step=512):
    return [(f, min(step, total - f)) for f in range(0, total, step)]


def build_nc():
    nc = bacc.Bacc(None)
    x_d = nc.dram_tensor("x", [SPC, C, N, T], F32, kind="ExternalInput")
    y_d = nc.dram_tensor("y", [SPC, C, N, T], F32, kind="ExternalOutput")
    convwTb_d = nc.dram_tensor("convwTb", [C, C], BF16, kind="ExternalInput")
    convb4_d = nc.dram_tensor("convb4", [1, 512], BF16, kind="ExternalInput")
    convb12p_d = nc.dram_tensor("convb12p", [C, 1], F32, kind="ExternalInput")
    memb_d = nc.dram_tensor("memb", [C, N + 1], BF16, kind="ExternalInput")
    fcw0_d = nc.dram_tensor("fcw0", [C, 1], F32, kind="ExternalInput")
    fcw1_d = nc.dram_tensor("fcw1", [C, 1], F32, kind="ExternalInput")
    fcb_d = nc.dram_tensor("fcb", [C, 1], F32, kind="ExternalInput")
    gw1Te_d = nc.dram_tensor("gw1Te", [C, C], BF16, kind="ExternalInput")
    gw2Te_d = nc.dram_tensor("gw2Te", [C, C], BF16, kind="ExternalInput")
    gbe_d = nc.dram_tensor("gbe", [C, 1], F32, kind="ExternalInput")
    if DBG:
        hT8_dbg = nc.dram_tensor("hT8_dbg", [SPC, C, NPR, 2, T, C], DT8, kind="ExternalOutput")
        xs_dbg = nc.dram_tensor("xs_dbg", [SPC, C, N + 1], BF16, kind="ExternalOutput")
        hs_dbg = nc.dram_tensor("hs_dbg", [SPC, C, N + 1], BF16, kind="ExternalOutput")
        adjP_dbg = nc.dram_tensor("adjP_dbg", [SPC, C, NPR, 2, N], DT8, kind="ExternalOutput")
        g1c_dbg = nc.dram_tensor("g1c_dbg", [SPC, C, N, T], BF16, kind="ExternalOutput")
        g2c_dbg = nc.dram_tensor("g2c_dbg", [SPC, C, N, T], BF16, kind="ExternalOutput")

    with tile.TileContext(nc) as tc:
        with (
            tc.tile_pool(name="const", bufs=1) as constp,
            tc.tile_pool(name="pers", bufs=2) as pers,
            tc.tile_pool(name="gper", bufs=1) as gper,
            tc.tile_pool(name="xin", bufs=2) as xinp,
            tc.tile_pool(name="x2in", bufs=2) as x2p,
            tc.tile_pool(name="scr", bufs=8 if DT8 == FP8 else 4) as scrp,
            tc.tile_pool(name="col", bufs=8) as colp,
            tc.tile_pool(name="outw", bufs=2) as outwp,
            tc.tile_pool(name="mmps", bufs=3, space=bass.MemorySpace.PSUM) as mmps,
            tc.tile_pool(name="dps", bufs=2, space=bass.MemorySpace.PSUM) as dpsp,
            tc.tile_pool(name="tpps", bufs=1, space=bass.MemorySpace.PSUM) as tpps,
        ):
            # ---- constants / weights ----
            identb = constp.tile([128, 128], BF16)
            make_identity(nc, identb[:])
            zeros = constp.tile([128, N], F32)
            nc.gpsimd.memset(zeros[:], 0.0)
            convwTb = constp.tile_from(convwTb_d[:])
            convb4 = constp.tile_from(convb4_d[:])
            ones1 = constp.tile([1, 128], BF16)
            nc.gpsimd.memset(ones1[:], 1.0)
            convb12p = constp.tile_from(convb12p_d[:])
            memb = constp.tile_from(memb_d[:])
            fcw0 = constp.tile_from(fcw0_d[:])
            fcw1 = constp.tile_from(fcw1_d[:])
            fcb = constp.tile_from(fcb_d[:])
            gw1Te = constp.tile_from(gw1Te_d[:])
            gw2Te = constp.tile_from(gw2Te_d[:])
            gbe = constp.tile_from(gbe_d[:])

            def stage_A(s):
                """conv t-planes -> hT8 (fp8 x8, pair-packed); xsumb; hsumb."""
                hT8 = pers.tile([128, NPR, 2, T, C], DT8, tag="hT8")
                xsumb = pers.tile([128, N + 1], BF16, tag="xsumb")
                hsumb = pers.tile([128, N + 1], BF16, tag="hsumb")
                # zero fp8 padding (pair 3 slot 1 fully; chunk-6 partitions)
                nc.gpsimd.memset(hT8[:, NPR - 1, 1], 0.0)
                nc.gpsimd.memset(hT8[96:, NPR - 1, 0], 0.0)
                nc.vector.memset(xsumb[:, N : N + 1], 0.0)

                xf = x_d[s].rearrange("c n t -> c (n t)")
                for j, (n0, sz) in enumerate(CH):
                    xb = xinp.tile([128, CT], BF16, tag="xb")
                    nc.gpsimd.dma_start(
                        xb[:, : sz * T], xf[:, n0 * T : (n0 + sz) * T]
                    )
                    xv = xb[:, : sz * T].rearrange("p (n t) -> p n t", t=T)
                    with nc.allow_low_precision(reason="bf16 xsum validated"):
                        nc.vector.tensor_reduce(
                            xsumb[:, n0 : n0 + sz], xv, axis=AX.X, op=OP.add
                        )
                    for tg in range(3):  # 4 t-planes per PSUM tile
                        ps = mmps.tile([128, 512], F32, tag="mm")
                        # bias as rank-1 matmul: ps = 1_n (x) conv_b (4x tiled)
                        nc.tensor.matmul(
                            ps[:sz], ones1[:, :sz], convb4[:],
                            start=True, stop=False, skip_group_check=True,
                        )
                        for tt in range(4):
                            t = tg * 4 + tt
                            nc.tensor.matmul(
                                ps[:sz, tt * 128 : (tt + 1) * 128],
                                xv[:, :, t], convwTb[:],
                                start=False, stop=True, skip_group_check=True,
                            )
                        dst = hT8[:sz, j // 2, j % 2, tg * 4 : tg * 4 + 4, :]
                        dv = dst.rearrange("p t c -> p (t c)")
                        nc.scalar.activation(dv, ps[:sz], ACTF.Copy, scale=S_H)

                # hsum = W @ xsum + T*conv_b (bf16; virtual col stays 0)
                for f0, fs in MCH:
                    ps = mmps.tile([128, 512], F32, tag="mm")
                    nc.tensor.matmul(
                        ps[:, :fs], convwTb[:], xsumb[:, f0 : f0 + fs],
                        start=True, stop=True,
                    )
                    real = min(fs, N - f0)  # exclude virtual col from bias
                    nc.vector.tensor_scalar(
                        hsumb[:, f0 : f0 + real], ps[:, :real], convb12p[:],
                        None, op0=OP.add,
                    )
                    if real < fs:
                        nc.scalar.activation(
                            hsumb[:, f0 + real : f0 + fs], ps[:, real:fs],
                            ACTF.Copy,
                        )
                if DBG:
                    nc.sync.dma_start(hT8_dbg[s], hT8[:])
                    nc.sync.dma_start(xs_dbg[s], xsumb[:])
                    nc.sync.dma_start(hs_dbg[s], hsumb[:])
                return hT8, xsumb, hsumb

            def stage_B(s, hsumb, filler=None):
                """adjacency + exact top-k mask -> adjP fp8 x256 pair-packed"""
                adjP = pers.tile([128, NPR, 2, NP8], DT8, tag="adjP")
                nc.gpsimd.memset(adjP[:, NPR - 1, 1], 0.0)
                nc.gpsimd.memset(adjP[96:, NPR - 1, 0], 0.0)
                nc.gpsimd.memset(adjP[:, :, :, N:], 0.0)

                for j, (n0, sz) in enumerate(CH):
                    if filler is not None:
                        filler(j)
                    lhs = hsumb[:, n0 : n0 + sz]
                    r1 = scrp.tile([128, N + 1], F32, tag="scr")
                    r2 = scrp.tile([128, N + 1], F32, tag="scr")
                    for (f0, fs), rt, rhs in (
                        (MCH[0], r1, memb), (MCH[1], r1, memb),
                        (MCH[0], r2, hsumb), (MCH[1], r2, hsumb),
                    ):
                        ps = mmps.tile([128, 512], F32, tag="mm")
                        nc.tensor.matmul(
                            ps[:sz, :fs], lhs, rhs[:, f0 : f0 + fs],
                            start=True, stop=True,
                        )
                        # relu(s * scale) -- matches reference op order
                        nc.scalar.activation(
                            rt[:sz, f0 : f0 + fs], ps[:sz, :fs], ACTF.Relu,
                            scale=SCALE,
                        )

                    def softmax_ext(rin, sz=sz):
                        """in-place softmax over cols [0,N); col N rides along.
                        No max-subtraction: s <= ~80 so exp stays in f32 range,
                        and ties still collapse to exp(0)=1 exactly."""
                        acc = colp.tile([128, 1], F32, tag="acc")
                        nc.scalar.activation(
                            rin[:sz], rin[:sz], ACTF.Exp,
                            scale=1.0, accum_out=acc[:sz],
                        )
                        zf = colp.tile([128, 1], F32, tag="zf")
                        nc.vector.tensor_sub(zf[:sz], acc[:sz], rin[:sz, N : N + 1])
                        nc.gpsimd.normalize_recip(rin[:sz], rin[:sz], zf[:sz])

                    softmax_ext(r1)  # r1 -> a1
                    softmax_ext(r2)  # r2 -> a2
                    # l = (fcw0*a1 + fcw1*a2) + fcb  -- reference association
                    t2 = scrp.tile([128, N + 1], F32, tag="scr")
                    nc.scalar.activation(
                        t2[:sz], r2[:sz], ACTF.Copy, scale=fcw1[:sz]
                    )
                    nc.vector.scalar_tensor_tensor(
                        r1[:sz], r1[:sz], fcw0[:sz], t2[:sz],
                        op0=OP.mult, op1=OP.add,
                    )
                    # (fcb add dropped: softmax is shift-invariant and the
                    # uniform shift preserves tie-group equality)
                    softmax_ext(r1)  # r1 -> adj
                    adj = r1
                    thr = adj[:sz, N : N + 1]
                    # ---- top-k mask, lowest-index tie breaking ----
                    gt = t2  # reuse
                    cnt = colp.tile([128, 1], F32, tag="cnt")
                    nc.vector.tensor_scalar(
                        gt[:sz, :N], adj[:sz, :N], thr, 0.0,
                        op0=OP.is_gt, op1=OP.add, accum_out=cnt[:sz],
                    )
                    eq = r2  # reuse
                    nc.vector.tensor_scalar(
                        eq[:sz, :N], adj[:sz, :N], thr, None, op0=OP.is_equal
                    )
                    # cum = cnt + prefix(eq); keep tie entries while cum <= K
                    cum = scrp.tile([128, N + 1], F32, tag="scr")
                    nc.vector.tensor_tensor_scan(
                        cum[:sz, :N], eq[:sz, :N], zeros[:sz, :N],
                        initial=cnt[:sz], op0=OP.add, op1=OP.add,
                    )
                    # eq <- (cum <= K)*eq ; then eq <- eq + gt
                    nc.vector.scalar_tensor_tensor(
                        eq[:sz, :N], cum[:sz, :N], float(K), eq[:sz, :N],
                        op0=OP.is_le, op1=OP.mult,
                    )
                    nc.vector.tensor_add(eq[:sz, :N], eq[:sz, :N], gt[:sz, :N])
                    # adjP = (adj * S_A) * mask, fp8
                    nc.vector.scalar_tensor_tensor(
                        adjP[:sz, j // 2, j % 2, :N], adj[:sz, :N], S_A,
                        eq[:sz, :N], op0=OP.mult, op1=OP.mult,
                    )
                if DBG:
                    nc.sync.dma_start(adjP_dbg[s], adjP[:])
                return adjP

            def scale_evict(i, dst, src, scl):
                if i % 2 == 0:
                    nc.vector.tensor_scalar(dst, src, scl, None, op0=OP.mult)
                else:
                    nc.scalar.activation(dst, src, ACTF.Copy, scale=scl)

            def emit_d_group(hT8, adjP, dstc, t, ei):
                """one diffusion t-plane: DoubleRow fp8 -> dstc[:, :, t]"""
                ps = dpsp.tile([128, NP8], F32, tag="dps")
                for m0, msp, ms in DCH_DR:
                    if USE_DR:
                        for jj in range(NPR):
                            nc.tensor.matmul(
                                ps[:, m0 : m0 + msp],
                                hT8[:, jj, :, t, :],
                                adjP[:, jj, :, m0 : m0 + msp],
                                start=(jj == 0), stop=(jj == NPR - 1),
                                perf_mode=DR,
                            )
                    else:
                        for jc in range(NCH):
                            nc.tensor.matmul(
                                ps[:, m0 : m0 + ms],
                                hT8[:, jc // 2, jc % 2, t, :],
                                adjP[:, jc // 2, jc % 2, m0 : m0 + ms],
                                start=(jc == 0), stop=(jc == NCH - 1),
                            )
                scale_evict(
                    ei + t, dstc[:, :, t], ps[:, :N], 1.0 / (S_H * S_A)
                )

            def stage_D(hT8, adjP, dstc, ei):
                for t in range(T):
                    emit_d_group(hT8, adjP, dstc, t, ei)

            def stage_T(g1c, g1T8):
                """transpose g1c -> g1T8 (fp8 x8, pair-packed)"""
                nc.gpsimd.memset(g1T8[:, NPR - 1, 1], 0.0)
                nc.gpsimd.memset(g1T8[96:, NPR - 1, 0], 0.0)
                for kk, (m0, msz) in enumerate(CH):
                    for tg in range(3):
                        tp = tpps.tile([128, 4, 128], BF16, tag="tp")
                        for tt in range(4):
                            t = tg * 4 + tt
                            nc.tensor.transpose(
                                tp[:msz, tt, :], g1c[:, m0 : m0 + msz, t],
                                identb[:],
                            )
                        dst = g1T8[:msz, kk // 2, kk % 2, tg * 4 : tg * 4 + 4, :]
                        scale_evict(
                            kk + tg,
                            dst.rearrange("p t c -> p (t c)"),
                            tp[:msz].rearrange("p t c -> p (t c)"), S_H,
                        )

            def stage_P(s, g1c, g2c):
                """projection (emb folded into weights) + skip + output DMA"""
                if DBG:
                    nc.sync.dma_start(g1c_dbg[s], g1c[:])
                    nc.sync.dma_start(g2c_dbg[s], g2c[:])
                xf = x_d[s].rearrange("c n t -> c (n t)")
                yf = y_d[s].rearrange("c n t -> c (n t)")
                g1f = g1c.rearrange("p n t -> p (n t)")
                g2f = g2c.rearrange("p n t -> p (n t)")
                for j, (n0, sz) in enumerate(CH):
                    ow = outwp.tile([128, CT], F32, tag="ow")
                    x2 = x2p.tile([128, CT], F32, tag="x2")
                    nc.sync.dma_start(
                        x2[:, : sz * T], xf[:, n0 * T : (n0 + sz) * T]
                    )
                    for f0, fs in _fch(sz * T):
                        ps = mmps.tile([128, 512], F32, tag="mm")
                        nc.tensor.matmul(
                            ps[:, :fs], gw1Te[:],
                            g1f[:, n0 * T + f0 : n0 * T + f0 + fs],
                            start=True, stop=False,
                        )
                        nc.tensor.matmul(
                            ps[:, :fs], gw2Te[:],
                            g2f[:, n0 * T + f0 : n0 * T + f0 + fs],
                            start=False, stop=True,
                        )
                        nc.vector.scalar_tensor_tensor(
                            ow[:, f0 : f0 + fs], ps[:, :fs], gbe[:],
                            x2[:, f0 : f0 + fs], op0=OP.add, op1=OP.add,
                        )
                    nc.sync.dma_start(yf[:, n0 * T : (n0 + sz) * T], ow[:, : sz * T])

            # ---- software-pipelined main loop ----
            prev = None
            for s in range(SPC):
                hT8, xsumb, hsumb = stage_A(s)
                if prev is not None:
                    ps_, hT8_, adjP_ = prev
                    g1c = gper.tile([128, N, T], BF16, tag="g1c")
                    g2c = gper.tile([128, N, T], BF16, tag="g2c")
                    g1T8 = gper.tile([128, NPR, 2, T, C], DT8, tag="g1T8")
                    stage_D(hT8_, adjP_, g1c, 0)
                    stage_T(g1c, g1T8)

                    def filler(j, g1T8=g1T8, adjP_=adjP_, g2c=g2c):
                        lo = (T * j) // NCH
                        hi = (T * (j + 1)) // NCH
                        for t in range(lo, hi):
                            emit_d_group(g1T8, adjP_, g2c, t, 1)
                else:
                    filler = None
                adjP = stage_B(s, hsumb, filler)
                if prev is not None:
                    stage_P(ps_, g1c, g2c)
                prev = (s, hT8, adjP)

            ps_, hT8_, adjP_ = prev
            g1c = gper.tile([128, N, T], BF16, tag="g1c")
            g2c = gper.tile([128, N, T], BF16, tag="g2c")
            g1T8 = gper.tile([128, NPR, 2, T, C], DT8, tag="g1T8")
            stage_D(hT8_, adjP_, g1c, 0)
            stage_T(g1c, g1T8)
            stage_D(g1T8, adjP_, g2c, 1)
            stage_P(ps_, g1c, g2c)
    nc.compile()
    return nc


_NC = None


def _get_nc():
    global _NC
    if _NC is None:
        _NC = build_nc()
    return _NC


def make_in_maps(inputs):
    x = np.ascontiguousarray(np.asarray(inputs["x"], dtype=np.float32))
    conv_w = np.asarray(inputs["conv_w"], np.float32)
    conv_b = np.asarray(inputs["conv_b"], np.float32)
    memory = np.asarray(inputs["memory"], np.float32)
    fc_w = np.asarray(inputs["fc_w"], np.float32)
    fc_b = np.asarray(inputs["fc_b"], np.float32)
    gcn_w = np.asarray(inputs["gcn_w"], np.float32)
    gcn_b = np.asarray(inputs["gcn_b"], np.float32)
    emb = np.asarray(inputs["emb"], np.float32).reshape(C)

    membx = np.zeros((C, N + 1), np.float32)
    membx[:, :N] = memory
    shared = {
        "convwTb": np.ascontiguousarray(conv_w.T).astype(ml_dtypes.bfloat16),
        "convb4": np.tile(conv_b[None, :], (1, 4)).astype(ml_dtypes.bfloat16),
        "convb12p": (T * conv_b).reshape(C, 1).copy(),
        "memb": membx.astype(ml_dtypes.bfloat16),
        "fcw0": np.full((C, 1), fc_w[0, 0], np.float32),
        "fcw1": np.full((C, 1), fc_w[0, 1], np.float32),
        "fcb": np.full((C, 1), fc_b[0], np.float32),
        "gw1Te": np.ascontiguousarray(
            (gcn_w[:, :C] * emb[:, None]).T
        ).astype(ml_dtypes.bfloat16),
        "gw2Te": np.ascontiguousarray(
            (gcn_w[:, C:] * emb[:, None]).T
        ).astype(ml_dtypes.bfloat16),
        "gbe": (gcn_b * emb).reshape(C, 1).astype(np.float32),
    }
    return [
        {"x": np.ascontiguousarray(x[c * SPC : (c + 1) * SPC]), **shared}
        for c in range(NCORES)
    ]


def kernel(**inputs) -> np.ndarray:
    nc = _get_nc()
    in_maps = make_in_maps(inputs)
    res = run_bass_kernel_spmd(nc, in_maps, list(range(NCORES)))
    outs = [res.results[c]["y"] for c in range(NCORES)]
    return np.concatenate(outs, axis=0).astype(np.float32)
